# revision 1
# baseline (speedup 1.0000x reference)
"""EventSpecificTimingHeads Trainium2 kernel (8 NeuronCores, SPMD).

Shards the E=16 independent per-event attention+MLP heads across 8 cores
(2 events per core). Each core computes logits[e, b, s] for its 2 events
over the full shared feature tensor; the host gathers and transposes to
[B, S, E].

Math per event e:
  qkv = x @ Wqkv[e].T + bqkv[e]  (q pre-scaled by 1/sqrt(Dh) via weights)
  per (b, h):  S.T = k q.T  (j, i layout);  P.T = exp(S.T)  (no max-sub:
  |scores| <~ 2 so exp is safe; softmax is shift-free mathematically)
  unnormalized pv.T = [v | 1].T @ P.T   -> ctx rows + row-sum l rows
  ctx.T = pv.T / l (per head), via transpose -> per-partition reciprocal
  -> column-broadcast multiply -> transpose back
  attended.T = Wo[e].T-as-lhsT @ ctx.T        (bv, bo folded into c1)
  h1 = relu(W1 attended + c1), c1 = W1(Wo bv + bo) + b1
  logits = w2_aug.T @ [h1; 1]                 (b2 folded into w2_aug)
"""
import sys

if "/opt/trn_rl_repo" not in sys.path:
    sys.path.insert(0, "/opt/trn_rl_repo")

import numpy as np
import ml_dtypes

import concourse.bass as bass
import concourse.bacc as bacc
import concourse.tile as tile
from concourse import mybir
from concourse import masks
from concourse.bass_utils import run_bass_kernel_spmd

BF16 = mybir.dt.bfloat16
F32 = mybir.dt.float32
AF = mybir.ActivationFunctionType
ALU = mybir.AluOpType

E, D, B, S, H, Dh, H2 = 16, 128, 8, 512, 4, 32, 64
T = B * S            # 4096
EV = 2               # events per core
NCORES = 8

_CACHED_NC = None


def build_nc():
    nc = bacc.Bacc(None, target_bir_lowering=False, debug=False)

    xT_d = nc.declare_dram_parameter("xT", [D, T], BF16, isOutput=False)
    wqkvT_d = nc.declare_dram_parameter("wqkvT", [D, EV, 3, D], BF16, isOutput=False)
    bqk_d = nc.declare_dram_parameter("bqk", [D, EV, 2], F32, isOutput=False)
    wfT_d = nc.declare_dram_parameter("wfT", [D, EV, H2], BF16, isOutput=False)
    c1b_d = nc.declare_dram_parameter("c1b", [H2, EV], F32, isOutput=False)
    w2a_d = nc.declare_dram_parameter("w2a", [H2 + 1, EV], BF16, isOutput=False)
    out_d = nc.declare_dram_parameter("out", [EV, B, S], F32, isOutput=True)

    with tile.TileContext(nc) as tc:
        with (
            tc.tile_pool(name="single", bufs=1) as single,
            tc.tile_pool(name="work", bufs=2) as work,
            tc.tile_pool(name="stp", bufs=2, space="PSUM") as stp,
            tc.tile_pool(name="pvp", bufs=2, space="PSUM") as pvp,
            tc.tile_pool(name="misc", bufs=2, space="PSUM") as misc,
        ):
            # ---- resident SBUF tensors ----
            xT_sb = single.tile([D, T], BF16)
            wqkvT_sb = single.tile([D, EV, 3, D], BF16)
            bqk_sb = single.tile([D, EV, 2], F32)
            wfT_sb = single.tile([D, EV, H2], BF16)
            c1b_sb = single.tile([H2, EV], F32)
            w2a_sb = single.tile([H2 + 1, EV], BF16)
            ident = single.tile([D, D], BF16)
            qT_sb = single.tile([D, EV, T], BF16)
            kT_sb = single.tile([D, EV, T], BF16)
            # v_aug: [j-in-chunk, ev, b, jc, h, 33]; col 32 of each h-block = 1.0
            v_sb = single.tile([D, EV, B, 4, H, Dh + 1], BF16)
            h1a_sb = single.tile([H2 + 1, 2, S], BF16)  # double-buffered, row 64 = ones

            masks.make_identity(nc, ident[:])
            # needed-first weights on sync; bulk xT on the scalar queue (idle
            # until the first exp); late-use weights last
            nc.sync.dma_start(out=wqkvT_sb[:], in_=wqkvT_d[:])
            nc.sync.dma_start(out=bqk_sb[:], in_=bqk_d[:])
            for n in range(8):
                nc.scalar.dma_start(out=xT_sb[:, n * S:(n + 1) * S],
                                    in_=xT_d[:, n * S:(n + 1) * S])
            nc.sync.dma_start(out=wfT_sb[:], in_=wfT_d[:])
            nc.sync.dma_start(out=c1b_sb[:], in_=c1b_d[:])
            nc.sync.dma_start(out=w2a_sb[:], in_=w2a_d[:])
            nc.gpsimd.memset(v_sb[:, :, :, :, :, Dh:Dh + 1], 1.0)
            nc.gpsimd.memset(h1a_sb[H2:H2 + 1, :, :], 1.0)

            # ---- q/k projection chunk; chunks are emitted lazily, two
            # batches ahead of consumption, so the first QK^T isn't stuck
            # behind all 32 projection matmuls in PE priority order
            def proj_chunk(n):
                for ev in range(EV):
                    for qk in range(2):
                        dst = qT_sb if qk == 0 else kT_sb
                        ps = misc.tile([D, S], F32, name="proj_ps", tag="m")
                        nc.tensor.matmul(
                            ps[:],
                            wqkvT_sb[:, ev, qk, :],
                            xT_sb[:, n * S:(n + 1) * S],
                        )
                        nc.vector.tensor_scalar_add(
                            dst[:, ev, n * S:(n + 1) * S],
                            ps[:],
                            bqk_sb[:, ev, qk:qk + 1],
                        )

            for n in range(8):
                proj_chunk(n)

            def project_v(b):
                # both events at once: rhs [128, 2*128], two t-chunks per psum
                for half in range(2):
                    psv = pvp.tile([D, S], F32, name="vproj_ps", tag="pv")
                    for c2 in range(2):
                        tch = 4 * b + 2 * half + c2
                        nc.tensor.matmul(
                            psv[:, c2 * 256:(c2 + 1) * 256],
                            xT_sb[:, tch * D:(tch + 1) * D],
                            wqkvT_sb[:, :, 2, :],
                        )
                    # psum col c2*256 + ev*128 + 32h + dh
                    for ev2 in range(EV):
                        nc.vector.tensor_copy(
                            v_sb[:, ev2, b, 2 * half:2 * half + 2, :, 0:Dh],
                            psv[:].rearrange(
                                "p (c e h d) -> p c e h d", c=2, e=2, h=H
                            )[:, :, ev2, :, :],
                        )

            # ---- main per-(event, batch) pipeline, software-pipelined:
            # head(i) = QK+exp, then tail(i-1) (always-ready transpose/MLP
            # work that keeps the PE dense while ACT chews the new scores),
            # then mid(i) = PV + psum drains.
            def emit_head(ev, b, pt):
                t0 = b * S
                for jc in range(4):
                    sts = [stp.tile([D, 2 * S], F32, name=f"st{hp}", tag="st")
                           for hp in range(2)]
                    for h in range(H):
                        nc.tensor.matmul(
                            sts[h // 2][:, (h % 2) * S:(h % 2 + 1) * S],
                            kT_sb[32 * h:32 * h + 32, ev,
                                  t0 + jc * D:t0 + (jc + 1) * D],
                            qT_sb[32 * h:32 * h + 32, ev, t0:t0 + S],
                            tile_position=(32 * h, 0),
                        )
                    for hp in range(2):
                        nc.scalar.activation(
                            pt[:, jc, 2 * hp:2 * hp + 2, :], sts[hp][:], AF.Exp
                        )

            def emit_mid(ev, b, pt):
                pva = pvp.tile([D, S], F32, name="pva", tag="pv")
                pvb = pvp.tile([D, S], F32, name="pvb", tag="pv")
                for jc in range(4):
                    for pk, pvt in ((0, pva), (1, pvb)):
                        for s2 in range(2):
                            h = 2 * pk + s2
                            nc.tensor.matmul(
                                pvt[64 * s2:64 * s2 + 33, :],
                                v_sb[:, ev, b, jc, h, :],
                                pt[:, jc, h, :],
                                start=(jc == 0),
                                stop=(jc == 3),
                                tile_position=(0, 64 * s2),
                            )
                pv_sb = work.tile([D, 2, S], BF16, name="pv_sb")
                nc.vector.tensor_copy(pv_sb[:, 0, :], pva[:])
                nc.vector.tensor_copy(pv_sb[:, 1, :], pvb[:])
                return pv_sb

            def emit_tail(ev, b, pv_sb):
                eb = ev * B + b
                ct0 = misc.tile([D, 2 * 256], BF16, name="ct0", tag="m")
                ct1 = misc.tile([D, 2 * 256], BF16, name="ct1", tag="m")
                for it in range(4):
                    ct = ct0 if it < 2 else ct1
                    for pk in range(2):
                        nc.tensor.transpose(
                            ct[:, (it % 2) * 256 + pk * D:
                               (it % 2) * 256 + pk * D + D],
                            pv_sb[:, pk, it * D:(it + 1) * D],
                            ident[:],
                        )
                linv = work.tile([D, 16], F32, name="linv")
                nc.vector.reciprocal(linv[:, 0:8], ct0[:, 32::64])
                nc.vector.reciprocal(linv[:, 8:16], ct1[:, 32::64])
                # scale ctx columns by 1/l and compact to [i, 4h*32]
                ctxn = work.tile([D, 4, H, Dh], BF16, name="ctxn")
                for t2 in range(2):
                    ct = ct0 if t2 == 0 else ct1
                    nc.vector.tensor_tensor(
                        ctxn[:, 2 * t2:2 * t2 + 2, :, :],
                        ct[:].rearrange("p (x y z) -> p x y z", x=2, y=4)[
                            :, :, :, 0:Dh],
                        linv[:].rearrange("p (x y) -> p x y", x=4)[
                            :, 2 * t2:2 * t2 + 2, :, None].to_broadcast(
                            [D, 2, H, Dh]),
                        ALU.mult,
                    )
                # transpose back to ctx.T [d, i]
                ctp = misc.tile([D, S], BF16, name="ctp", tag="m")
                for it in range(4):
                    nc.tensor.transpose(
                        ctp[:, it * D:(it + 1) * D],
                        ctxn[:, it, :, :],
                        ident[:],
                    )
                ctxT = work.tile([D, S], BF16, name="ctxT")
                nc.vector.tensor_copy(ctxT[:], ctp[:])
                # fused (W1 @ Wo) + relu(g + c1)
                gp = misc.tile([H2, S], F32, name="gp", tag="m")
                nc.tensor.matmul(gp[:], wfT_sb[:, ev, :], ctxT[:])
                nc.vector.tensor_scalar(
                    h1a_sb[0:H2, eb % 2, :],
                    gp[:],
                    c1b_sb[:, ev:ev + 1],
                    0.0,
                    ALU.add,
                    ALU.max,
                )
                # W2 (aug) and store
                lgp = misc.tile([1, S], F32, name="lgp", tag="m")
                nc.tensor.matmul(
                    lgp[:], w2a_sb[:, ev:ev + 1], h1a_sb[:, eb % 2, :]
                )
                lg_sb = work.tile([1, S], F32, name="lg_sb")
                nc.vector.tensor_copy(lg_sb[:], lgp[:])
                nc.sync.dma_start(out=out_d[ev, b, :], in_=lg_sb[0:1, :])

            project_v(0)
            prev = None
            for ev in range(EV):
                for b in range(B):
                    pt = work.tile([D, 4, H, S], BF16, name="pt", bufs=4)
                    emit_head(ev, b, pt)
                    if prev is not None:
                        emit_tail(*prev)
                    pv_sb = emit_mid(ev, b, pt)
                    if ev == 0 and b + 1 < B:
                        project_v(b + 1)
                    prev = (ev, b, pv_sb)
            emit_tail(*prev)

    nc.compile()
    return nc


def _prep_inputs(lstm_features, Wqkv, bqkv, Wo, bo, W1, b1, W2, b2):
    """Host-side per-core input prep (numpy, fp32 -> bf16 where PE-facing)."""
    bf = ml_dtypes.bfloat16
    x = np.asarray(lstm_features, np.float32).reshape(T, D)
    xT = np.ascontiguousarray(x.T).astype(bf)
    scale = 1.0 / np.sqrt(np.float32(Dh))

    in_maps = []
    for c in range(NCORES):
        evs = [2 * c, 2 * c + 1]
        wqkvT = np.zeros((D, EV, 3, D), np.float32)
        bqk = np.zeros((D, EV, 2), np.float32)
        wfT = np.zeros((D, EV, H2), np.float32)
        c1b = np.zeros((H2, EV), np.float32)
        w2a = np.zeros((H2 + 1, EV), np.float32)
        for i, e in enumerate(evs):
            Wq = Wqkv[e, 0:D, :] * scale
            Wk = Wqkv[e, D:2 * D, :]
            Wv = Wqkv[e, 2 * D:3 * D, :]
            wqkvT[:, i, 0, :] = Wq.T
            wqkvT[:, i, 1, :] = Wk.T
            wqkvT[:, i, 2, :] = Wv.T
            bqk[:, i, 0] = bqkv[e, 0:D] * scale
            bqk[:, i, 1] = bqkv[e, D:2 * D]
            bv = bqkv[e, 2 * D:3 * D]
            bo_eff = Wo[e] @ bv + bo[e]
            wfT[:, i, :] = (W1[e] @ Wo[e]).T
            c1b[:, i] = W1[e] @ bo_eff + b1[e]
            w2a[0:H2, i] = W2[e, 0, :]
            w2a[H2, i] = b2[e, 0]
        in_maps.append({
            "xT": xT,
            "wqkvT": wqkvT.astype(bf),
            "bqk": bqk,
            "wfT": wfT.astype(bf),
            "c1b": c1b,
            "w2a": w2a.astype(bf),
        })
    return in_maps


def kernel(lstm_features, Wqkv, bqkv, Wo, bo, W1, b1, W2, b2, _trace=False):
    global _CACHED_NC
    args = [np.asarray(a, np.float32) for a in
            (lstm_features, Wqkv, bqkv, Wo, bo, W1, b1, W2, b2)]
    in_maps = _prep_inputs(*args)
    if _CACHED_NC is None:
        _CACHED_NC = build_nc()
    res = run_bass_kernel_spmd(
        _CACHED_NC, in_maps, list(range(NCORES)), trace=_trace
    )
    logits = np.concatenate(
        [np.asarray(res.results[c]["out"], np.float32) for c in range(NCORES)],
        axis=0,
    )  # [16, 8, 512]
    out = np.ascontiguousarray(logits.transpose(1, 2, 0))  # [B, S, E]
    if _trace:
        return out, res
    return out

